# revision 5
# baseline (speedup 1.0000x reference)
"""NefClass fuzzy-rule classifier kernel v4 for 8x Trainium2 NeuronCores.

Math: out[b,c] = sum_{r: class[r]=c} min_f relu_mem[f, cond[r,f], b]
where relu_mem = min(relu(w1 x + b1), relu(p2 x + b2)) (relu commutes with
min, so clipping memberships at 0 at the source makes every downstream
value >= 0 and no final relu is needed).

Per core (batch-sharded 8 ways, 2048 cols each):
  1. x -> x_rep [112, 2048] via 7 DMAs (split across two DMA rings);
     memberships via ACT Relu-affine (left) in parallel with DVE
     tensor_scalar affine+relu (right), then DVE min.
  2. Pair tables: 4 packed [128, 2048] bf16 tiles, 2 feature-pair groups per
     tile (49-row combo blocks at partition bases 0 and 64). Built by PE
     one-hot replication matmuls (L chunks then R chunks to minimize
     LDWEIGHTS), drained by double ACT copy + DVE bf16 min. Tables 1,2,3
     (groups 2..7) go to DRAM for the indirect gathers; table 0 is built
     last and only used by the PE gathers.
  3. Rule firing per 128-rule tile: groups 0,1 PE-gathered and ACT-copied
     to bf16 leaves; groups 2..7 gathered by per-tile indirect DMA rows
     from the DRAM tables. Merged by a DVE bf16 min chain.
  4. Class segment-sum via one-hot class matmul accumulating in PSUM.
"""

import numpy as np
import ml_dtypes

import concourse.bass as bass
import concourse.mybir as mybir
import concourse.tile as tile
from concourse.bass_utils import run_bass_kernel_spmd

F = 16          # features
M = 7           # membership functions per feature
C = 10          # classes
R = 512         # rules
B = 16384       # batch
NCORES = 8
BL = B // NCORES     # 2048 batch per core
FM = F * M           # 112
RT = R // 128        # 4 rule tiles of 128 rules
G = F // 2           # 8 pair groups
NP = G // 2          # 4 packed table tiles (2 groups per tile)
MM2 = M * M          # 49 combos per pair
N_PE = 2             # groups gathered via PE one-hot matmul (0..1)
N_DMA = G - N_PE     # groups gathered via indirect DMA (2..7)
HB = 1024            # psum tile width (2 banks)

F32 = mybir.dt.float32
BF16 = mybir.dt.bfloat16
BF16_NP = ml_dtypes.bfloat16

AF = mybir.ActivationFunctionType
ALU = mybir.AluOpType

_PROGRAM = None


def _split_multi_waits(nc):
    """Walrus codegen only encodes ONE sem wait per instruction; hoist extra
    waits into standalone NOPs on the same engine."""
    k = 0
    for fn in nc.m.functions:
        for blk in fn.blocks:
            old = list(blk.instructions)
            new = []
            changed = False
            for ins in old:
                si = getattr(ins, "sync_info", None)
                eng = getattr(ins, "engine", None)
                if si is not None and len(si.on_wait) > 1 and eng is not None:
                    waits = list(si.on_wait)
                    for w in waits[:-1]:
                        nop = mybir.InstNoOp(
                            name=f"{ins.name}_ws{k}",
                            sync_info=mybir.SyncInfo(on_wait=[w], on_update=[]),
                            bass_nofuse=True,
                            engine=eng,
                        )
                        k += 1
                        new.append(nop)
                    ins.sync_info = mybir.SyncInfo(
                        on_wait=[waits[-1]], on_update=list(si.on_update)
                    )
                    changed = True
                new.append(ins)
            if changed:
                blk.instructions = new


def _build_program():
    nc = bass.Bass("TRN2", target_bir_lowering=False)

    x_d = nc.dram_tensor("x", [F, BL], F32, kind="ExternalInput").ap()
    prm_d = nc.dram_tensor("prm", [FM, 4], F32, kind="ExternalInput").ap()
    rl_d = nc.dram_tensor("rl", [FM, NP * 128], BF16, kind="ExternalInput").ap()
    rr_d = nc.dram_tensor("rr", [FM, NP * 128], BF16, kind="ExternalInput").ap()
    gp_d = nc.dram_tensor("gp", [128, N_PE * RT * 128], BF16,
                          kind="ExternalInput").ap()
    ch_d = nc.dram_tensor("ch", [128, RT * C], BF16, kind="ExternalInput").ap()
    idx_d = nc.dram_tensor("idx", [128, N_DMA * RT], mybir.dt.int32,
                           kind="ExternalInput").ap()
    out_d = nc.dram_tensor("out", [C, BL], F32, kind="ExternalOutput").ap()
    tabds = {p: nc.dram_tensor(f"tabd{p}", [128, BL], BF16).ap()
             for p in (1, 2, 3)}

    with tile.TileContext(nc) as tc:
        with (
            tc.tile_pool(name="const", bufs=1) as constp,
            tc.tile_pool(name="xrp", bufs=1) as xrp,
            tc.tile_pool(name="lr", bufs=1) as lrp,
            tc.tile_pool(name="memp", bufs=1) as memp,
            tc.tile_pool(name="tabp", bufs=1) as tabp,
            tc.tile_pool(name="clh", bufs=2) as clhp,
            tc.tile_pool(name="leaf", bufs=1) as leafp,
            tc.tile_pool(name="dgp", bufs=1) as dgp,
            tc.tile_pool(name="chain", bufs=3) as chp,
            tc.tile_pool(name="fire", bufs=1) as firep,
            tc.tile_pool(name="outp", bufs=1) as outp,
        ):
            # ---- input DMAs: x replication split across both rings ----
            prm = constp.tile([FM, 4], F32)
            nc.sync.dma_start(prm[:], prm_d[:])
            xr = xrp.tile([FM, BL], F32)
            xr3 = xr[:].rearrange("(f m) b -> f m b", m=M)
            for m in range(M):
                if m % 2 == 0:
                    nc.sync.dma_start(xr3[:, m, :], x_d[:, :])
                else:
                    nc.scalar.dma_start(xr3[:, m, :], x_d[:, :])
            rl = constp.tile([FM, NP * 128], BF16)
            nc.scalar.dma_start(rl[:], rl_d[:])
            rr = constp.tile([FM, NP * 128], BF16)
            nc.scalar.dma_start(rr[:], rr_d[:])
            gp = constp.tile([128, N_PE * RT * 128], BF16)
            nc.scalar.dma_start(gp[:], gp_d[:])
            idx = constp.tile([128, N_DMA * RT], mybir.dt.int32)
            nc.sync.dma_start(idx[:], idx_d[:])
            ch = constp.tile([128, RT * C], BF16)
            nc.sync.dma_start(ch[:], ch_d[:])

            # ---- memberships (relu folded in; all values >= 0) ----
            warm = lrp.tile([FM, 4], F32, tag="warm")
            nc.scalar.activation(warm[:], prm[:], AF.Relu)
            left = lrp.tile([FM, BL], BF16, tag="left")
            nc.scalar.activation(
                left[:], xr[:], AF.Relu, scale=prm[:, 0:1], bias=prm[:, 1:2]
            )
            right = lrp.tile([FM, BL], BF16, tag="right")
            nc.vector.tensor_scalar(
                out=right[:], in0=xr[:], scalar1=prm[:, 2:3],
                scalar2=prm[:, 3:4], op0=ALU.mult, op1=ALU.add,
            )
            mem = memp.tile([FM, BL], BF16)
            nc.vector.scalar_tensor_tensor(
                out=mem[:], in0=right[:], scalar=0.0, in1=left[:],
                op0=ALU.max, op1=ALU.min,
            )

            # ---- pair tables (DMA-side tables 1,2,3 first) ----
            tables = {}
            tabps = tc.tile_pool(name="pstab", bufs=2, space="PSUM")
            psp = tabps.__enter__()
            for p in (1, 2, 3, 0):
                tab = tabp.tile([128, BL], BF16, tag=f"tab{p}")
                tables[p] = tab
                for h in range(BL // HB):
                    hs = slice(HB * h, HB * (h + 1))
                    psl = psp.tile([128, HB], F32, tag="psA")
                    psr = psp.tile([128, HB], F32, tag="psB")
                    for q in range(HB // 512):
                        sl = slice(HB * h + 512 * q, HB * h + 512 * (q + 1))
                        nc.tensor.matmul(
                            out=psl[:, 512 * q: 512 * (q + 1)],
                            lhsT=rl[:, 128 * p: 128 * (p + 1)],
                            rhs=mem[:, sl], start=True, stop=True,
                        )
                    for q in range(HB // 512):
                        sl = slice(HB * h + 512 * q, HB * h + 512 * (q + 1))
                        nc.tensor.matmul(
                            out=psr[:, 512 * q: 512 * (q + 1)],
                            lhsT=rr[:, 128 * p: 128 * (p + 1)],
                            rhs=mem[:, sl], start=True, stop=True,
                        )
                    cl = clhp.tile([128, HB], BF16, tag="cl")
                    nc.scalar.activation(cl[:], psl[:], AF.Copy)
                    cr = clhp.tile([128, HB], BF16, tag="cr")
                    nc.scalar.activation(cr[:], psr[:], AF.Copy)
                    nc.vector.tensor_tensor(
                        out=tab[:, hs], in0=cl[:], in1=cr[:], op=ALU.min
                    )
                if p in (1, 2, 3):
                    nc.sync.dma_start(tabds[p][:, :], tab[:])
            tabps.__exit__(None, None, None)

            # ---- indirect DMA gathers for groups 2..7 (per rule tile) ----
            dgs = {}
            for g in range(N_PE, G):
                for t in range(RT):
                    dg = dgp.tile([128, BL], BF16, tag=f"dg{g}_{t}")
                    dgs[(g, t)] = dg
                    col = (g - N_PE) * RT + t
                    nc.gpsimd.indirect_dma_start(
                        out=dg[:], out_offset=None,
                        in_=tabds[g // 2][:, :],
                        in_offset=bass.IndirectOffsetOnAxis(
                            ap=idx[:, col: col + 1], axis=0
                        ),
                    )

            # ---- PE gathers for groups 0,1 + ACT-leaf drains + tree ----
            firing = []
            for t in range(RT):
                fir = firep.tile([128, BL], BF16, tag=f"fir{t}")
                firing.append(fir)

            gps_ctx = tc.tile_pool(name="psg", bufs=2, space="PSUM")
            psgp = gps_ctx.__enter__()
            for t in range(RT):
                leaves = []
                for g in range(N_PE):
                    cg = leafp.tile([128, BL], BF16, tag=f"c{g}")
                    leaves.append(cg)
                    base = 64 * (g % 2)
                    rhs_tab = tables[g // 2][base: base + MM2, :]
                    lhsT = gp[
                        base: base + MM2,
                        (g * RT + t) * 128: (g * RT + t + 1) * 128,
                    ]
                    for h in range(BL // HB):
                        ps = psgp.tile([128, HB], F32, tag=f"pg{g}")
                        for q in range(HB // 512):
                            sl = slice(HB * h + 512 * q, HB * h + 512 * (q + 1))
                            nc.tensor.matmul(
                                out=ps[:, 512 * q: 512 * (q + 1)], lhsT=lhsT,
                                rhs=rhs_tab[:, sl], start=True, stop=True,
                            )
                        nc.scalar.activation(
                            cg[:, HB * h: HB * (h + 1)], ps[:], AF.Copy
                        )

                # DMA-side pair tree matching gather arrival order
                a = chp.tile([128, BL], BF16, tag="x")
                nc.vector.tensor_tensor(out=a[:], in0=dgs[(2, t)][:],
                                        in1=dgs[(3, t)][:], op=ALU.min)
                b = chp.tile([128, BL], BF16, tag="x")
                nc.vector.tensor_tensor(out=b[:], in0=dgs[(4, t)][:],
                                        in1=dgs[(5, t)][:], op=ALU.min)
                y = chp.tile([128, BL], BF16, tag="y")
                nc.vector.tensor_tensor(out=y[:], in0=leaves[0][:],
                                        in1=leaves[1][:], op=ALU.min)
                ab = chp.tile([128, BL], BF16, tag="x")
                nc.vector.tensor_tensor(out=ab[:], in0=a[:], in1=b[:],
                                        op=ALU.min)
                aby = chp.tile([128, BL], BF16, tag="y")
                nc.vector.tensor_tensor(out=aby[:], in0=ab[:], in1=y[:],
                                        op=ALU.min)
                c = chp.tile([128, BL], BF16, tag="x")
                nc.vector.tensor_tensor(out=c[:], in0=dgs[(6, t)][:],
                                        in1=dgs[(7, t)][:], op=ALU.min)
                nc.vector.tensor_tensor(out=firing[t][:], in0=aby[:], in1=c[:],
                                        op=ALU.min)
            gps_ctx.__exit__(None, None, None)

            # ---- class segment-sum ----
            outs = outp.tile([C, BL], F32)
            cls_ctx = tc.tile_pool(name="pscl", bufs=2, space="PSUM")
            psclp = cls_ctx.__enter__()
            for h in range(2):
                psc = psclp.tile([128, HB], F32, tag="psc")
                for q in range(2):
                    off = 1024 * h + 512 * q
                    for t in range(RT):
                        nc.tensor.matmul(
                            out=psc[0:C, 512 * q: 512 * (q + 1)],
                            lhsT=ch[:, t * C: (t + 1) * C],
                            rhs=firing[t][:, off: off + 512],
                            start=(t == 0),
                            stop=(t == RT - 1),
                        )
                nc.scalar.activation(
                    outs[:, 1024 * h: 1024 * (h + 1)], psc[0:C, :], AF.Copy
                )
            cls_ctx.__exit__(None, None, None)
            nc.sync.dma_start(out_d[:], outs[:])

    _split_multi_waits(nc)
    return nc


def _host_inputs(x, mf_abc, rule_conditions, rule_classes):
    x = np.ascontiguousarray(np.asarray(x, dtype=np.float32))
    abc = np.asarray(mf_abc, dtype=np.float32).reshape(FM, 3)
    cond = np.asarray(rule_conditions).astype(np.int64)
    cls = np.asarray(rule_classes).astype(np.int64)

    a, b_, c_ = abc[:, 0], abc[:, 1], abc[:, 2]
    w1 = 1.0 / (b_ - a)
    p2 = -1.0 / (c_ - b_)
    prm = np.stack([w1, -a * w1, p2, -c_ * p2], axis=1).astype(np.float32)

    rl = np.zeros([FM, NP, 128], dtype=BF16_NP)
    rr = np.zeros([FM, NP, 128], dtype=BF16_NP)
    j49 = np.arange(MM2)
    for p in range(NP):
        rl[4 * p * M + j49 // M, p, j49] = 1
        rr[(4 * p + 1) * M + j49 % M, p, j49] = 1
        rl[(4 * p + 2) * M + j49 // M, p, 64 + j49] = 1
        rr[(4 * p + 3) * M + j49 % M, p, 64 + j49] = 1
    rl = np.ascontiguousarray(rl.reshape(FM, NP * 128))
    rr = np.ascontiguousarray(rr.reshape(FM, NP * 128))

    j = np.arange(R)
    t_idx, jj = j // 128, j % 128
    gpm = np.zeros([128, N_PE, RT, 128], dtype=BF16_NP)
    for g in range(N_PE):
        combo = cond[:, 2 * g] * M + cond[:, 2 * g + 1] + 64 * (g % 2)
        gpm[combo, g, t_idx, jj] = 1
    gpm = np.ascontiguousarray(gpm.reshape(128, N_PE * RT * 128))

    chm = np.zeros([128, RT, C], dtype=BF16_NP)
    chm[jj, t_idx, cls] = 1
    chm = np.ascontiguousarray(chm.reshape(128, RT * C))

    idx = np.zeros([128, N_DMA, RT], dtype=np.int32)
    for g in range(N_PE, G):
        combo = cond[:, 2 * g] * M + cond[:, 2 * g + 1]
        idx[jj, g - N_PE, t_idx] = 64 * (g % 2) + combo
    idx = np.ascontiguousarray(idx.reshape(128, N_DMA * RT))

    return x, prm, rl, rr, gpm, chm, idx


def kernel(x, mf_abc, rule_conditions, rule_classes):
    global _PROGRAM
    if _PROGRAM is None:
        _PROGRAM = _build_program()

    xf, prm, rl, rr, gpm, chm, idx = _host_inputs(
        x, mf_abc, rule_conditions, rule_classes
    )

    in_maps = [
        {
            "x": np.ascontiguousarray(xf[:, i * BL: (i + 1) * BL]),
            "prm": prm,
            "rl": rl,
            "rr": rr,
            "gp": gpm,
            "ch": chm,
            "idx": idx,
        }
        for i in range(NCORES)
    ]
    res = run_bass_kernel_spmd(_PROGRAM, in_maps, core_ids=list(range(NCORES)))
    out = np.concatenate([r["out"].T for r in res.results], axis=0)
    return np.ascontiguousarray(out.astype(np.float32))
